# revision 1
# baseline (speedup 1.0000x reference)
"""Segment-reduce (min/max/mean per contiguous span) on 8 Trainium2 cores.

Sharding: pure data parallel — core b handles batch b (the 8 per-core
programs differ only in the span boundaries baked into them; span_idxs is
host data when kernel() is called, so the programs are specialized at build
time).

Per-core algorithm, in feature-major ("chunk-major") layout
A[p, 4096c + t] = x[t, 128c + p]:

- DVE computes, per 1024-token piece q and per chunk c, one masked
  tensor_tensor_scan for min and one for max: state = (mask + state) op x,
  where mask is +/-BIG at span starts — the running min/max restarts at every
  span, so the scan value at a span's last token IS that span's min/max.
  It also computes one 3-D windowed sum-reduce per span ([128, 4chunk, L]).
- Extraction of the scan values at span-end columns (one [128, 4]
  chunk-strided copy per span per stat) runs on the Scalar and GpSimd
  engines, overlapped with DVE's next piece. Spans that straddle a piece
  boundary get a single fused tensor_tensor(min/max) against the previous
  piece's last scan column (GpSimd), and their sums are head+tail partial
  reduces combined with one add.
- mean = sum * (1/L) as one [128, 1024] multiply against a host reciprocal
  tile.

Execution: under axon there is no native NRT path, so each specialized
program is dispatched to its own NeuronCore via the same PJRT custom-call
primitive run_bass_kernel_spmd uses (run_bass_via_pjrt's single-core path),
pinned with jax.default_device.
"""

import sys
import threading

sys.path.insert(0, "/opt/trn_rl_repo")

import numpy as np

B, T, D, S = 8, 4096, 512, 256
PT = 1024  # tokens per piece
NP = T // PT
BIG = 3.0e38


def _spans(span_starts):
    # Reference segments tokens by searchsorted(starts) — span s covers
    # [starts[s], starts[s+1]-1] (last span runs to T-1).
    starts = span_starts.astype(np.int64)
    ends = np.empty_like(starts)
    ends[:-1] = starts[1:] - 1
    ends[-1] = T - 1
    return starts, ends


def _build_program(starts, ends):
    import concourse.bass as bass
    import concourse.mybir as mybir

    f32 = mybir.dt.float32
    bf16 = mybir.dt.bfloat16
    X = mybir.AxisListType.X
    Alu = mybir.AluOpType
    nc = bass.Bass(target_bir_lowering=False)
    A = nc.dram_tensor("A", [128, 4 * T], f32, kind="ExternalInput")
    R = nc.dram_tensor("R", [128, 4 * S], f32, kind="ExternalInput")
    MK = nc.dram_tensor("MK", [128, 3 * T], bf16, kind="ExternalInput")
    OUT = nc.dram_tensor("OUT", [128, 8 * S], f32, kind="ExternalOutput")
    ME = nc.dram_tensor("ME", [128, 4 * S], f32, kind="ExternalOutput")

    piece_of = [int(e // PT) for e in ends]
    spans_in = [[s for s in range(S) if piece_of[s] == q] for q in range(NP)]
    straddler = [int(starts[s]) < PT * piece_of[s] for s in range(S)]
    # GP is the faster copier in the cost model (~101ns vs ACT ~190ns): GP
    # takes all straddlers (they need tensor_tensor) plus ~65% of the rest.
    gp_spans, act_spans = [[] for _ in range(NP)], [[] for _ in range(NP)]
    for q in range(NP):
        plain = [s for s in spans_in[q] if not straddler[s]]
        k = (len(plain) * 13) // 20
        gp_spans[q] = [s for s in spans_in[q] if straddler[s]] + plain[:k]
        act_spans[q] = plain[k:]

    from contextlib import ExitStack

    with ExitStack() as ctx:
        block = ctx.enter_context(nc.Block())
        sem = lambda n: ctx.enter_context(nc.semaphore(n))
        sb = lambda n, shape, dt: ctx.enter_context(nc.sbuf_tensor(n, shape, dt))
        r_sem = sem("r_sem")
        mk_sems = [sem(f"mk{i}_sem") for i in range(NP)]
        ap_sems = [sem(f"ap{i}_sem") for i in range(NP)]
        scn_sem = sem("scn_sem")
        p0_sems = [sem(f"p0c{i}_sem") for i in range(4)]
        eact_sem, egp_sem = sem("eact_sem"), sem("egp_sem")
        v_sem, o_sem = sem("v_sem"), sem("o_sem")
        Ap0 = sb("Ap0", [128, 4 * PT], f32)
        Ap1 = sb("Ap1", [128, 4 * PT], f32)
        Ap2 = sb("Ap2", [128, 4 * PT], f32)
        STo0 = sb("STo0", [128, 12 * PT], f32)
        STo1 = sb("STo1", [128, 12 * PT], f32)
        MKb = sb("MKb", [128, 3 * T], bf16)
        MKmn = MKb[:, :T]
        MKmx = MKb[:, T:]
        R_sb = sb("R_sb", [128, 4 * S], f32)
        OUT_sb = sb("OUT_sb", [128, 12 * S], f32)
        ME_sb = sb("ME_sb", [128, 4 * S], f32)
        Ap = [Ap0, Ap1, Ap2]
        STo = [STo0, STo1]
        SM_slice = OUT_sb[:, 8 * S : 12 * S]

        def span_seg(s, lo, hi):
            """[128, 4, hi-lo] view of tokens [lo, hi) in the resident piece."""
            q = lo // PT
            off = lo - PT * q
            return (
                Ap[q % 2][:]
                .rearrange("p (c t) -> p c t", c=4)[:, :, off : off + (hi - lo)]
            )

        def stat_col(slot, sigma, col):
            """[128, 4] chunk-strided view of one stat's scan column."""
            return (
                STo[slot][:, 4 * PT * sigma : 4 * PT * (sigma + 1)]
                .rearrange("p (c t) -> p c t", c=4)[:, :, col]
            )

        def all_stats_col(slot, col):
            """[128, 3, 4] view: all 3 stats x 4 chunks at one column."""
            return STo[slot][:].rearrange("p (g c t) -> p g c t", g=3, c=4)[
                :, :, :, col
            ]

        def out_span(s):
            """[128, 3, 4] destination in OUT_sb for span s."""
            return OUT_sb[:].rearrange("p (g s4) -> p g s4", g=3)[
                :, :, 4 * s : 4 * s + 4
            ]

        @block.gpsimd
        def _(g):
            g.dma_start(
                MKb[:, : 3 * PT], MK[:, : 3 * PT]
            ).then_inc(mk_sems[0], 16)
            g.dma_start(
                MKb[:, 3 * PT : 6 * PT], MK[:, 3 * PT : 6 * PT]
            ).then_inc(mk_sems[1], 16)
            g.dma_start(R_sb[:], R[:]).then_inc(r_sem, 16)
            for q in range(NP):
                if q + 2 < NP:
                    g.dma_start(
                        MKb[:, 3 * PT * (q + 2) : 3 * PT * (q + 3)],
                        MK[:, 3 * PT * (q + 2) : 3 * PT * (q + 3)],
                    ).then_inc(mk_sems[q + 2], 16)
                g.wait_ge(scn_sem, q + 1)
                glast = [None]
                for s in gp_spans[q]:
                    if straddler[s]:
                        continue
                    b = int(ends[s]) - PT * q
                    glast[0] = g.tensor_copy(out_span(s), all_stats_col(q % 2, b))
                glast[0].then_inc(egp_sem, 1) if glast[0] is not None else g.sem_inc(egp_sem, 1)
                if spans_in[q]:
                    g.wait_ge(eact_sem, q + 1)
                    g.wait_ge(egp_sem, q + 1)
                    s0, s1 = spans_in[q][0], spans_in[q][-1] + 1
                    rng = (
                        OUT_sb[:, : 8 * S]
                        .rearrange("p (g s4) -> p g s4", g=2)[
                            :, :, 4 * s0 : 4 * s1
                        ]
                    )
                    rng_d = (
                        OUT[:]
                        .rearrange("p (g s4) -> p g s4", g=2)[
                            :, :, 4 * s0 : 4 * s1
                        ]
                    )
                    g.dma_start(rng_d, rng).then_inc(o_sem, 16)
            g.wait_ge(v_sem, 1)
            g.dma_start(ME[:], ME_sb[:]).then_inc(o_sem, 16)
            g.wait_ge(o_sem, 16 * (1 + sum(1 for q in range(NP) if spans_in[q])))

        @block.sync
        def _(sy):
            for c in range(4):
                sy.dma_start(
                    Ap[0][:, PT * c : PT * (c + 1)],
                    A[:, 4096 * c : 4096 * c + PT],
                ).then_inc(p0_sems[c], 16)
            for q in range(1, NP):
                if q >= 3:
                    # A slot q%3 is free once DVE's scans of piece q-3 are done
                    sy.wait_ge(scn_sem, q - 2)
                sy.dma_start(
                    Ap[q % 3][:].rearrange("p (c t) -> p c t", c=4),
                    A[:].rearrange("p (c t) -> p c t", c=4)[
                        :, :, PT * q : PT * (q + 1)
                    ],
                ).then_inc(ap_sems[q], 16)

        @block.scalar
        def _(sc):
            for q in range(NP):
                sc.wait_ge(scn_sem, q + 1)
                alast = None
                for s in act_spans[q]:
                    b = int(ends[s]) - PT * q
                    alast = nc.scalar.copy(out_span(s), all_stats_col(q % 2, b))
                if alast is not None:
                    alast.then_inc(eact_sem, 1)
                else:
                    sc.sem_inc(eact_sem, 1)

        @block.vector
        def _(v):
            for q in range(NP):
                v.wait_ge(mk_sems[q], 16)
                if q > 0:
                    v.wait_ge(ap_sems[q], 16)
                if q >= 2:
                    # scan-out slot q%2 is reused from piece q-2 (bulk
                    # extraction) and read by GP's straddler at piece q-1.
                    v.wait_ge(eact_sem, q - 1)
                    v.wait_ge(egp_sem, q - 1)
                for c in range(4):
                    if q == 0:
                        v.wait_ge(p0_sems[c], 16)
                    xs = Ap[q % 3][:, PT * c : PT * (c + 1)]
                    for sigma, (m0, init, op0, op1) in enumerate(
                        (
                            (0, BIG, Alu.add, Alu.min),
                            (PT, -BIG, Alu.add, Alu.max),
                            (2 * PT, 0.0, Alu.mult, Alu.add),
                        )
                    ):
                        nc.vector.tensor_tensor_scan(
                            STo[q % 2][
                                :, 4 * PT * sigma + PT * c : 4 * PT * sigma + PT * (c + 1)
                            ],
                            MKb[:, 3 * PT * q + m0 : 3 * PT * q + m0 + PT],
                            xs,
                            init,
                            op0=op0,
                            op1=op1,
                        )
                v.drain()
                vstr = None
                for s in spans_in[q]:
                    if not straddler[s]:
                        continue
                    b = int(ends[s]) - PT * q
                    for sigma, op in ((0, Alu.min), (1, Alu.max), (2, Alu.add)):
                        vstr = nc.vector.tensor_tensor(
                            OUT_sb[
                                :, 4 * S * sigma + 4 * s : 4 * S * sigma + 4 * s + 4
                            ],
                            stat_col(q % 2, sigma, b),
                            stat_col((q - 1) % 2, sigma, PT - 1),
                            op,
                        )
                if vstr is not None:
                    v.drain()
                v.sem_inc(scn_sem, 1)

            v.wait_ge(r_sem, 16)
            v.wait_ge(eact_sem, NP - 1)
            v.wait_ge(egp_sem, NP - 1)
            sE = spans_in[NP - 1][0] if spans_in[NP - 1] else S
            if sE > 0:
                nc.vector.tensor_mul(
                    ME_sb[:, : 4 * sE],
                    OUT_sb[:, 8 * S : 8 * S + 4 * sE],
                    R_sb[:, : 4 * sE],
                )
            v.wait_ge(eact_sem, NP)
            v.wait_ge(egp_sem, NP)
            nc.vector.tensor_mul(
                ME_sb[:, 4 * sE :],
                OUT_sb[:, 8 * S + 4 * sE : 12 * S],
                R_sb[:, 4 * sE :],
            ).then_inc(v_sem, 1)

    return nc


class CoreRunner:
    """jit-once runner for one specialized program on one NeuronCore.

    Mirrors bass2jax.run_bass_via_pjrt's single-core path but keeps the
    jitted callable so repeated executions don't re-lower/re-compile.
    """

    def __init__(self, nc, device, core_id):
        import jax
        import concourse.mybir as mybir
        from concourse.bass2jax import install_neuronx_cc_hook, _bass_exec_p

        install_neuronx_cc_hook()
        self.device = device
        self.core_id = core_id
        self.pid_name = (
            nc.partition_id_tensor.name if nc.partition_id_tensor is not None else None
        )
        self.in_names = []
        self.out_names = []
        out_avals = []
        self.zero_outs = []
        for alloc in nc.m.functions[0].allocations:
            if not isinstance(alloc, mybir.MemoryLocationSet):
                continue
            name = alloc.memorylocations[0].name
            if alloc.kind == "ExternalInput":
                self.in_names.append(name)
            elif alloc.kind == "ExternalOutput":
                self.out_names.append(name)
                shape = tuple(alloc.tensor_shape)
                dt = mybir.dt.np(alloc.dtype)
                out_avals.append(jax.core.ShapedArray(shape, dt))
                self.zero_outs.append(np.zeros(shape, dt))
        all_in = tuple(self.in_names + self.out_names)
        n_params = len(self.in_names)
        out_names = tuple(self.out_names)
        out_avals_t = tuple(out_avals)

        def _body(*args):
            return tuple(
                _bass_exec_p.bind(
                    *args,
                    out_avals=out_avals_t,
                    in_names=all_in,
                    out_names=out_names,
                    lowering_input_output_aliases=(),
                    sim_require_finite=False,
                    sim_require_nnan=False,
                    nc=nc,
                )
            )

        self._jit = jax.jit(
            _body, donate_argnums=tuple(range(n_params, n_params + len(out_names)))
        )

    def start(self, in_map):
        """Dispatch asynchronously; returns jax arrays."""
        import jax

        if self.pid_name is not None:
            in_map = {**in_map, self.pid_name: np.array([[self.core_id]], np.uint32)}
        with jax.default_device(self.device):
            args = [np.asarray(in_map[n]) for n in self.in_names] + [
                z.copy() for z in self.zero_outs
            ]
            return self._jit(*args)

    def finish(self, out_arrs):
        return {n: np.asarray(a) for n, a in zip(self.out_names, out_arrs)}


_RUNNERS = None
_RUNNER_META = None
_LOCK = threading.Lock()


def _get_runners(span_idxs):
    """Build + jit the 8 per-core programs (cached on span structure)."""
    global _RUNNERS, _RUNNER_META
    key = span_idxs.tobytes()
    with _LOCK:
        if _RUNNERS is not None and _RUNNER_META[0] == key:
            return _RUNNERS, _RUNNER_META[1]
        import jax

        devs = jax.devices()[:B]
        spans = [_spans(span_idxs[b, :, 0]) for b in range(B)]
        runners = []
        for b in range(B):
            nc = _build_program(*spans[b])
            runners.append(CoreRunner(nc, devs[b], b))
        _RUNNERS = runners
        _RUNNER_META = (key, spans)
        return runners, spans


def _pack_inputs(input, spans):
    in_maps = []
    import ml_dtypes

    for b in range(B):
        starts, ends = spans[b]
        A_b = np.ascontiguousarray(
            input[b].reshape(T, 4, 128).transpose(2, 1, 0).reshape(128, 4 * T)
        )
        lens = (ends - starts + 1).astype(np.float32)
        R_b = np.ascontiguousarray(
            np.broadcast_to(np.repeat(1.0 / lens, 4)[None, :], (128, 4 * S))
        )
        # piece-packed masks: MK[p, 3*PT*q + sigma*PT + t'] for piece q
        mn = np.zeros((T,), np.float32)
        mn[starts] = BIG
        sm = np.ones((T,), np.float32)
        sm[starts] = 0.0
        mk = np.stack([mn, -mn, sm], axis=0)  # [3, T]
        mk = (
            mk.reshape(3, NP, PT).transpose(1, 0, 2).reshape(3 * T)
        )  # [q][sigma][t']
        MK_b = np.ascontiguousarray(
            np.broadcast_to(mk[None, :], (128, 3 * T))
        ).astype(ml_dtypes.bfloat16)
        in_maps.append({"A": A_b, "R": R_b, "MK": MK_b})
    return in_maps


def _unpack(res_b):
    def fix(M):
        return M.reshape(128, S, 4).transpose(1, 2, 0).reshape(S, D)

    return np.concatenate(
        [
            fix(res_b["OUT"][:, : 4 * S]),
            fix(res_b["OUT"][:, 4 * S :]),
            fix(res_b["ME"]),
        ],
        axis=-1,
    )


def kernel(input, lengths, span_idxs):
    input = np.asarray(input, dtype=np.float32)
    lengths = np.asarray(lengths, dtype=np.int32)
    span_idxs = np.asarray(span_idxs, dtype=np.int32)

    runners, spans = _get_runners(span_idxs)
    in_maps = _pack_inputs(input, spans)

    pending = [None] * B

    def launch(b):
        pending[b] = runners[b].start(in_maps[b])

    threads = [threading.Thread(target=launch, args=(b,)) for b in range(B)]
    for t in threads:
        t.start()
    for t in threads:
        t.join()

    out = np.zeros((B, S, 3 * D), np.float32)
    for b in range(B):
        out[b] = _unpack(runners[b].finish(pending[b]))

    valid = ~((span_idxs[..., 0] == 0) & (span_idxs[..., 1] == 0)) & (
        np.arange(S)[None, :] < lengths[:, None]
    )
    out[~valid] = 0.0
    return out



# revision 50
# speedup vs baseline: 1.5582x; 1.5582x over previous
"""Segment-reduce (min/max/mean per contiguous span) on 8 Trainium2 cores.

Sharding: pure data parallel -- core b handles batch b. Programs are
specialized at build time on the span structure (span_idxs is host data).

Per-core algorithm (v2.1, fold-bucket design):

- min/max: host pads every span to lam = 2^ceil(lg L) rows (pad = repeat of
  the span's first element, neutral for min/max) and lays spans out in
  per-lam (sub-)buckets [lam, 4chunk, n_spans] in a feature-major layout
  (partition p = d % 128, chunk c = d // 128, bf16). Each sub-bucket is one
  DMA piece and one independent tensor_tensor fold-tree chain (bf16 2x DVE
  mode, 0.52 ns/elem), split between DVE and GPSIMD. Big chains stop at
  2-row remnants collected in a shared R2 array; one final TT per stat
  finishes all of them at once. Final values land contiguously in bucket
  order; the host un-permutes. No masks, no scans, no per-span extraction.
- sum/mean: TensorE matmul. lhsT = packed one-hot [128 tok, spans_in_tile]
  (fp8, ~10 cols per K-tile), rhs = x^T tile [128 tok, 512 d] (fp8),
  accumulating seg-sums in PSUM [s, d] (two banks for s 0-127 / 128-255,
  pre-zeroed by DVE). ACT scales by per-partition 1/L (activation Copy
  with scale vector) straight out of PSUM.
- spans with L <= 8 additionally get an exact bf16 fold-sum (fp8 error on
  tiny spans could breach tolerance): sum-fold over the padded rows, minus
  a host correction (lam-L)*x[start], times 1/L. Host takes mean for these
  spans from this path.

Outputs are bf16 (tolerance 2e-2); the host reassembles/permutes/casts.

Execution: each specialized program runs on its own NeuronCore via the
PJRT custom-call primitive (run_bass_via_pjrt's single-core path).
"""

import sys
import threading

sys.path.insert(0, "/opt/trn_rl_repo")

import numpy as np

B, T, D, S = 8, 4096, 512, 256
NK = T // 128  # matmul K-tiles
SUB_MAX = 4800  # max per-partition elems in one sub-bucket (DMA piece ~1.2MB)
GP_TARGET = 0  # fold elems (2 stats) assigned to GPSIMD (0 = GP disabled)
GP_EXTRA_LAMS = ()  # additional lam groups folded on GPSIMD
MEAN_ON_GP = False  # mean = psum * 1/L on GPSIMD instead of DVE


def _spans(span_starts):
    starts = span_starts.astype(np.int64)
    ends = np.empty_like(starts)
    ends[:-1] = starts[1:] - 1
    ends[-1] = T - 1
    return starts, ends


def _plan(starts, ends):
    """Bucket layout, sub-splitting, engine assignment, K-tile packing."""
    L = ends - starts + 1
    lam = np.maximum(2, 2 ** np.ceil(np.log2(np.maximum(L, 1))).astype(int))

    groups = {}
    for l in sorted(set(lam.tolist()), reverse=True):
        groups[l] = np.where(lam == l)[0]

    # GPSIMD takes the big lam-group whose 2-stat fold work (to 8-row
    # remnants) is closest to GP_TARGET. (GP custom tensor ops are not
    # supported by the axon lowering -- keep disabled until they are.)
    gp_lam = None
    best = None
    if GP_TARGET > 0:
        for l, spans in groups.items():
            if l < 16:
                continue
            work = 2 * 4 * len(spans) * (l - 8)
            score = abs(work - GP_TARGET)
            if best is None or score < best:
                best = score
                gp_lam = l

    # sub-bucket splitting; GP's first sub is kept small so GPSIMD can
    # start folding as soon as the first (small) DMA piece lands
    def make_subs(l, spans, gp):
        n = len(spans)
        if n == 0:
            return []
        subs = []
        i0 = 0
        if gp and n > 12:
            subs.append(spans[:8])
            i0 = 8
        max_n = max(2, SUB_MAX // (l * 4))
        rem = n - i0
        nsub = (rem + max_n - 1) // max_n
        per = (rem + nsub - 1) // nsub if nsub else rem
        for i in range(i0, n, per):
            subs.append(spans[i : i + per])
        # n >= 8 for big subs: keeps every fold width >= 64 elements
        # (narrower DVE tensor_tensor ops misbehave on this backend)
        return [
            dict(
                lam=l,
                spans=sp,
                nreal=len(sp),
                n=max(len(sp) + (len(sp) % 2), 8 if l >= 16 else 2),
                big=(l >= 16),
                gp=gp,
            )
            for sp in subs
        ]

    gp_lams = {gp_lam} | set(GP_EXTRA_LAMS) if gp_lam else set(GP_EXTRA_LAMS)
    gp_subs = []
    small_subs = []
    dve_big_subs = []
    for l, spans in groups.items():
        if l >= 16 and l in gp_lams:
            gp_subs.extend(make_subs(l, spans, True))
        elif l >= 16:
            dve_big_subs.extend(make_subs(l, spans, False))
        else:
            small_subs.extend(make_subs(l, spans, False))

    # APAD / DMA-piece order: GP data first, then smalls, then DVE bigs.
    order = gp_subs + small_subs + dve_big_subs
    off = 0
    for sb_ in order:
        sb_["off"] = off
        off += sb_["lam"] * 4 * sb_["n"]
    W = off

    # output columns: bigs (R2 order = their order in `order`), then smalls
    bigs = [s for s in order if s["big"]]
    smalls = [s for s in order if not s["big"]]
    NB = sum(s["n"] for s in bigs)
    NS = sum(s["n"] for s in smalls)
    SW = NB + NS
    col = 0
    for s in bigs:
        s["col"] = col  # also its R2 column offset
        col += s["n"]
    scol = 0
    for s in smalls:
        s["col"] = NB + scol
        s["s_off"] = scol
        scol += s["n"]
    perm = np.full(SW, -1, np.int64)
    for s in order:
        perm[s["col"] : s["col"] + s["nreal"]] = s["spans"]
    sperm = perm[NB:]

    # DMA pieces: one per big sub; all smalls together.
    # Transfer order (= SP issue order): interleave GP/DVE data so both
    # engines start early; AT (issued by ACT) lands mid-stream.
    pieces = []
    for s in gp_subs:
        pieces.append([s])
    if smalls:
        pieces.append(list(smalls))
    for s in dve_big_subs:
        pieces.append([s])
    for i, pc in enumerate(pieces):
        for s in pc:
            s["piece"] = i
    # issue order (sim-tuned): smalls first, then GP subs, then DVE big
    # groups by descending total elems
    issue = []
    if smalls:
        issue.append(smalls[0]["piece"])
    for s in gp_subs:
        issue.append(s["piece"])
    gsz = {}
    for s in dve_big_subs:
        gsz[s["lam"]] = gsz.get(s["lam"], 0) + s["lam"] * 4 * s["n"]
    for s in sorted(dve_big_subs, key=lambda s: -gsz[s["lam"]]):
        issue.append(s["piece"])
    seen = set()
    issue = [i for i in issue if not (i in seen or seen.add(i))]

    # token -> span id; K-tile one-hot packing (spans are the matmul free
    # dim, so no alignment constraints)
    seg = np.searchsorted(starts, np.arange(T), side="right") - 1
    ktiles = []
    oh_off = 0
    for q in range(NK):
        s_lo = int(seg[128 * q])
        s_hi = int(seg[128 * q + 127])
        m = s_hi - s_lo + 1
        ktiles.append(dict(s_lo=s_lo, m=m, off=oh_off))
        oh_off += m
    OHW = oh_off

    return dict(
        starts=starts,
        ends=ends,
        L=L,
        lam=lam,
        seg=seg,
        order=order,
        pieces=pieces,
        issue=issue,
        at_gate=issue[min(3, len(issue) - 1)],
        gp_subs=gp_subs,
        small_subs=smalls,
        dve_big_subs=dve_big_subs,
        W=W,
        NB=NB,
        NS=NS,
        SW=SW,
        perm=perm,
        sperm=sperm,
        ktiles=ktiles,
        OHW=OHW,
    )


def _build_program(plan):
    import concourse.bass as bass
    import concourse.mybir as mybir

    f32 = mybir.dt.float32
    bf16 = mybir.dt.bfloat16
    fp8 = mybir.dt.float8e4
    Alu = mybir.AluOpType
    Act = mybir.ActivationFunctionType
    nc = bass.Bass(target_bir_lowering=False)

    W, NB, NS, SW, OHW = plan["W"], plan["NB"], plan["NS"], plan["SW"], plan["OHW"]
    ktiles = plan["ktiles"]
    pieces = plan["pieces"]
    gp_subs = plan["gp_subs"]
    smalls = plan["small_subs"]
    dve_bigs = plan["dve_big_subs"]

    APAD = nc.dram_tensor("APAD", [128, W], bf16, kind="ExternalInput")
    AT = nc.dram_tensor("AT", [128, NK * D], fp8, kind="ExternalInput")
    OH = nc.dram_tensor("OH", [128, OHW], fp8, kind="ExternalInput")
    RC = nc.dram_tensor("RC", [128, S], bf16, kind="ExternalInput")
    CORRS = nc.dram_tensor("CORRS", [128, 8 * NS], bf16, kind="ExternalInput")
    # OUT planes: [min 4*SW | max 4*SW | smean 4*NS | mean 4*S], all d-major
    O_MIN, O_MAX = 0, 4 * SW
    O_SMEAN = 8 * SW
    O_ME = 8 * SW + 4 * NS
    OUTW = O_ME + 4 * S
    OUT = nc.dram_tensor("OUT", [128, OUTW], bf16, kind="ExternalOutput")

    from contextlib import ExitStack

    with ExitStack() as ctx:
        block = ctx.enter_context(nc.Block())
        sem = lambda n: ctx.enter_context(nc.semaphore(n))
        sb = lambda n, shape, dt: ctx.enter_context(nc.sbuf_tensor(n, shape, dt))

        psems = [sem(f"p{i}_sem") for i in range(len(pieces))]
        at_sems = [sem("at0_sem"), sem("at1_sem")]
        oh_sem = sem("oh_sem")
        rc_sem = sem("rc_sem")
        cs_sem = sem("cs_sem")
        psum_sem = sem("psum_sem")
        gp_rem = [sem("gp_rem0"), sem("gp_rem1")]
        min_done = sem("min_done")
        max_done = sem("max_done")
        mean_done = sem("mean_done")
        smean_done = sem("smean_done")
        o_sem = sem("o_sem")

        APAD_sb = sb("APAD_sb", [128, W], bf16)
        AT_sb = sb("AT_sb", [128, NK * D], fp8)
        OH_sb = sb("OH_sb", [128, OHW], fp8)
        RC_sb = sb("RC_sb", [128, S], bf16)
        CORRS_sb = sb("CORRS_sb", [128, 8 * NS], bf16)
        OUT_sb = sb("OUT_sb", [128, OUTW], bf16)
        R2 = [sb(f"R2_{s}", [128, 2 * 4 * NB], bf16) for s in range(2)]
        SS = sb("SS", [128, 4 * NS], bf16)
        SS2 = sb("SS2", [128, 4 * NS], bf16)
        P0 = ctx.enter_context(nc.psum_tensor("P0", [128, 512], f32))
        P1 = ctx.enter_context(nc.psum_tensor("P1", [128, 512], f32))

        # per-engine ping-pong fold scratch (reused chain to chain)
        def pool_sizes(subs, floor):
            a = b = 0
            for s_ in subs:
                szs = []
                rows = s_["lam"] // 2
                while rows >= floor:
                    szs.append(rows * 4 * s_["n"])
                    rows //= 2
                for i, sz in enumerate(szs):
                    if i % 2 == 0:
                        a = max(a, sz)
                    else:
                        b = max(b, sz)
            return a, b

        da, db = pool_sizes(dve_bigs + smalls, 1)
        da = max(da, 8 * 4 * max((s["n"] for s in gp_subs), default=0) // 2)
        ga, gb = pool_sizes(gp_subs, 8)
        DP = [sb("dpoolA", [128, max(da, 4)], bf16), sb("dpoolB", [128, max(db, 4)], bf16)]
        GPP = [sb("gpoolA", [128, max(ga, 4)], bf16), sb("gpoolB", [128, max(gb, 4)], bf16)]
        # persistent 8-row remnants for GP subs (read later by DVE)
        R8s = {}
        for sg in range(2):
            for i, s_ in enumerate(gp_subs):
                R8s[(sg, i)] = sb(f"r8_{sg}_{i}", [128, 8 * 4 * s_["n"]], bf16)

        def bview(s_):
            return (
                APAD_sb[:, s_["off"] : s_["off"] + s_["lam"] * 4 * s_["n"]]
                .rearrange("p (j c n) -> p j c n", j=s_["lam"], c=4)
            )

        def r2v(sg):
            return R2[sg][:].rearrange("p (j c n) -> p j c n", j=2, c=4)

        def out_cols(base, width, cols, n):
            return (
                OUT_sb[:, base : base + 4 * width]
                .rearrange("p (c w) -> p c w", c=4)[:, :, cols : cols + n]
            )

        OPS = {0: Alu.min, 1: Alu.max, 2: Alu.add}

        def fold_chain(eng, sg, s_, pool, cur=None, rows=None, stop_rows=None):
            """Fold [rows,4,n] by halving. Returns last instr.

            stop_rows=2 big chains write their last level into R2 columns;
            stop_rows=8 (GP) writes into the sub's persistent R8s buffer;
            stop_rows=1 (smalls / smallsum) writes OUT / SS.
            """
            op = OPS[sg]
            n = s_["n"]
            rw = 4 * n  # row width (elements) -- rows are contiguous
            if cur is None:
                cur = APAD_sb[:, s_["off"] : s_["off"] + s_["lam"] * rw]
                rows = s_["lam"]
            last = None
            pi = 0
            while rows > stop_rows:
                h = rows // 2
                in0 = cur[:, : h * rw]
                in1 = cur[:, h * rw : 2 * h * rw]
                if h == stop_rows and stop_rows == 2:
                    dst = r2v(sg)[:, :, :, s_["col"] : s_["col"] + n]
                    in0 = in0.rearrange("p (j c n) -> p j c n", j=2, c=4)
                    in1 = in1.rearrange("p (j c n) -> p j c n", j=2, c=4)
                elif h == stop_rows and stop_rows == 8:
                    dst = R8s[(sg, s_["gpi"])][:, : h * rw]
                elif h == 1:
                    if sg == 2:
                        dst = SS[:].rearrange("p (c n) -> p c n", c=4)[
                            :, :, s_["s_off"] : s_["s_off"] + n
                        ]
                    else:
                        dst = out_cols(O_MIN if sg == 0 else O_MAX, SW, s_["col"], n)
                    in0 = in0.rearrange("p (c n) -> p c n", c=4)
                    in1 = in1.rearrange("p (c n) -> p c n", c=4)
                else:
                    dst = pool[pi % 2][:, : h * rw]
                    pi += 1
                last = eng.tensor_tensor(dst, in0, in1, op)
                cur = dst
                rows = h
            return last

        for i, s_ in enumerate(gp_subs):
            s_["gpi"] = i

        @block.sync
        def _(sy):
            for i in plan["issue"]:
                pc = pieces[i]
                lo = pc[0]["off"]
                hi = pc[-1]["off"] + pc[-1]["lam"] * 4 * pc[-1]["n"]
                sy.dma_start(APAD_sb[:, lo:hi], APAD[:, lo:hi]).then_inc(psems[i], 16)

        @block.scalar
        def _(sc):
            sc.dma_start(OH_sb[:], OH[:]).then_inc(oh_sem, 16)
            sc.dma_start(RC_sb[:], RC[:]).then_inc(rc_sem, 16)
            sc.dma_start(CORRS_sb[:], CORRS[:]).then_inc(cs_sem, 16)
            # gate AT behind the early APAD pieces (PE only needs psum late)
            sc.wait_ge(psems[plan["at_gate"]], 16)
            sc.dma_start(AT_sb[:, : 16 * D], AT[:, : 16 * D]).then_inc(at_sems[0], 16)
            sc.dma_start(AT_sb[:, 16 * D :], AT[:, 16 * D :]).then_inc(at_sems[1], 16)
            sc.wait_ge(smean_done, 1)
            sc.dma_start(
                OUT[:, O_SMEAN : O_SMEAN + 4 * NS],
                OUT_sb[:, O_SMEAN : O_SMEAN + 4 * NS],
            ).then_inc(o_sem, 16)
            sc.wait_ge(min_done, 1)
            sc.dma_start(
                OUT[:, O_MIN : O_MIN + 4 * SW], OUT_sb[:, O_MIN : O_MIN + 4 * SW]
            ).then_inc(o_sem, 16)
            sc.wait_ge(mean_done, 1)
            sc.dma_start(
                OUT[:, O_ME : O_ME + 4 * S], OUT_sb[:, O_ME : O_ME + 4 * S]
            ).then_inc(o_sem, 16)
            sc.wait_ge(max_done, 1)
            sc.dma_start(
                OUT[:, O_MAX : O_MAX + 4 * SW], OUT_sb[:, O_MAX : O_MAX + 4 * SW]
            ).then_inc(o_sem, 16)
            sc.wait_ge(o_sem, 64)

        @block.tensor
        def _(pe):
            pe.wait_ge(oh_sem, 16)
            for half in range(2):
                pe.wait_ge(at_sems[half], 16)
                for q in range(16 * half, 16 * half + 16):
                    kt = ktiles[q]
                    for c in range(4):
                        P = P0 if c < 2 else P1
                        coloff = 256 * (c % 2)
                        is_last = q == NK - 1 and c % 2 == 1
                        mm = nc.tensor.matmul(
                            P[:, coloff + kt["s_lo"] : coloff + kt["s_lo"] + kt["m"]],
                            AT_sb[:, D * q + 128 * c : D * q + 128 * (c + 1)],
                            OH_sb[:, kt["off"] : kt["off"] + kt["m"]],
                            start=(q == 0 and c % 2 == 0),
                            stop=is_last,
                            skip_group_check=True,
                        )
                        if is_last:
                            mm.then_inc(psum_sem, 1)

        def emit_mean(eng):
            eng.wait_ge(psum_sem, 2)
            eng.wait_ge(rc_sem, 16)
            for c in range(4):
                P = P0 if c < 2 else P1
                coloff = 256 * (c % 2)
                last = eng.tensor_tensor(
                    out_cols(O_ME, S, 0, S)[:, c],
                    P[:, coloff : coloff + S],
                    RC_sb[:],
                    Alu.mult,
                )
            return last

        @block.gpsimd
        def _(g):
            for sg in range(2):
                last = None
                for s_ in gp_subs:
                    g.wait_ge(psems[s_["piece"]], 16)
                    last = fold_chain(g, sg, s_, GPP, stop_rows=8)
                if last is not None:
                    last.then_inc(gp_rem[sg], 1)
                else:
                    g.sem_inc(gp_rem[sg], 1)
            if MEAN_ON_GP:
                emit_mean(g).then_inc(mean_done, 1)

        @block.vector
        def _(v):
            # process pieces in DMA issue order; min+max per piece so late
            # pieces don't block early work
            for i in plan["issue"]:
                pc = pieces[i]
                if not pc[0]["big"]:
                    v.wait_ge(psems[i], 16)
                    for sg in (0, 1, 2):
                        for s_ in pc:
                            fold_chain(v, sg, s_, DP, stop_rows=1)
                    v.wait_ge(cs_sem, 16)
                    v.tensor_tensor(
                        SS2[:], SS[:], CORRS_sb[:, : 4 * NS], Alu.subtract
                    )
                    v.tensor_tensor(
                        OUT_sb[:, O_SMEAN : O_SMEAN + 4 * NS],
                        SS2[:],
                        CORRS_sb[:, 4 * NS :],
                        Alu.mult,
                    ).then_inc(smean_done, 1)
                elif not pc[0]["gp"]:
                    v.wait_ge(psems[i], 16)
                    for s_ in pc:
                        fold_chain(v, 0, s_, DP, stop_rows=2)
                        fold_chain(v, 1, s_, DP, stop_rows=2)
            # finish GP remnants 8 -> 2 into R2, then one final TT per stat
            for sg in (0, 1):
                if gp_subs:
                    v.wait_ge(gp_rem[sg], 1)
                    for s_ in gp_subs:
                        fold_chain(
                            v, sg, s_, DP,
                            cur=R8s[(sg, s_["gpi"])][:],
                            rows=8,
                            stop_rows=2,
                        )
                if NB:
                    base = O_MIN if sg == 0 else O_MAX
                    v.tensor_tensor(
                        out_cols(base, SW, 0, NB), r2v(sg)[:, 0], r2v(sg)[:, 1], OPS[sg]
                    )
                v.drain()
                v.sem_inc(min_done if sg == 0 else max_done, 1)

            # mean = psum * (1/L): [128 d, S] per chunk
            if not MEAN_ON_GP:
                emit_mean(v)
                v.drain()
                v.sem_inc(mean_done, 1)

    return nc


def _pack_inputs(input, plans):
    import ml_dtypes

    bf16 = ml_dtypes.bfloat16
    try:
        fp8 = ml_dtypes.float8_e4m3
    except AttributeError:
        fp8 = ml_dtypes.float8_e4m3fn

    in_maps = []
    for b in range(B):
        x = input[b]  # [T, D] f32
        plan = plans[b]
        W, NS, OHW = plan["W"], plan["NS"], plan["OHW"]
        starts, L = plan["starts"], plan["L"]

        APAD = np.zeros((128, W), np.float32)
        for bk in plan["order"]:
            lamk, n, nreal = bk["lam"], bk["n"], bk["nreal"]
            spans = bk["spans"]
            j = np.arange(lamk)
            tok = np.where(
                j[None, :] < L[spans][:, None],
                starts[spans][:, None] + j[None, :],
                starts[spans][:, None],
            )
            arr = x[tok]  # [nreal, lam, D]
            arr = arr.reshape(nreal, lamk, 4, 128).transpose(3, 1, 2, 0)
            dst = APAD[:, bk["off"] : bk["off"] + lamk * 4 * n].reshape(
                128, lamk, 4, n
            )
            dst[:, :, :, :nreal] = arr
        APAD = APAD.astype(bf16)

        AT = np.ascontiguousarray(
            x.reshape(NK, 128, D).transpose(1, 0, 2).reshape(128, NK * D)
        ).astype(fp8)

        OHm = np.zeros((128, OHW), np.float32)
        seg = plan["seg"]
        t = np.arange(128)
        for q, kt in enumerate(plan["ktiles"]):
            OHm[t, kt["off"] + seg[128 * q + t] - kt["s_lo"]] = 1.0
        OHm = OHm.astype(fp8)

        RC = np.ascontiguousarray(
            np.broadcast_to((1.0 / L.astype(np.float32))[None, :], (128, S))
        ).astype(bf16)

        CORRS = np.zeros((128, 8 * NS), np.float32)
        corr = CORRS[:, : 4 * NS].reshape(128, 4, NS)
        rcs = CORRS[:, 4 * NS :].reshape(128, 4, NS)
        for bk in plan["small_subs"]:
            spans = bk["spans"]
            pad = (bk["lam"] - L[spans]).astype(np.float32)
            x0 = x[starts[spans]]  # [nreal, D]
            cc = (pad[:, None] * x0).reshape(-1, 4, 128).transpose(2, 1, 0)
            sl = slice(bk["s_off"], bk["s_off"] + bk["nreal"])
            corr[:, :, sl] = cc
            rcs[:, :, sl] = (1.0 / L[spans].astype(np.float32))[None, None, :]
        CORRS = CORRS.astype(bf16)

        in_maps.append({"APAD": APAD, "AT": AT, "OH": OHm, "RC": RC, "CORRS": CORRS})
    return in_maps


def _unpack(res_b, plan):
    NB, NS, SW = plan["NB"], plan["NS"], plan["SW"]
    O = res_b["OUT"].astype(np.float32)
    O_MIN, O_MAX = 0, 4 * SW
    O_SMEAN = 8 * SW
    O_ME = 8 * SW + 4 * NS

    def plane(base, width):
        return (
            O[:, base : base + 4 * width]
            .reshape(128, 4, width)
            .transpose(2, 1, 0)
            .reshape(width, D)
        )

    out = np.zeros((S, 3 * D), np.float32)
    perm = plan["perm"]
    valid = perm >= 0
    out[perm[valid], 0:D] = plane(O_MIN, SW)[valid]
    out[perm[valid], D : 2 * D] = plane(O_MAX, SW)[valid]
    out[:, 2 * D :] = plane(O_ME, S)
    if NS:
        sperm = plan["sperm"]
        sv = sperm >= 0
        out[sperm[sv], 2 * D :] = plane(O_SMEAN, NS)[sv]
    return out


class CoreRunner:
    """jit-once runner for one specialized program on one NeuronCore."""

    def __init__(self, nc, device, core_id):
        import jax
        import concourse.mybir as mybir
        from concourse.bass2jax import install_neuronx_cc_hook, _bass_exec_p

        install_neuronx_cc_hook()
        self.device = device
        self.core_id = core_id
        self.pid_name = (
            nc.partition_id_tensor.name if nc.partition_id_tensor is not None else None
        )
        self.in_names = []
        self.out_names = []
        out_avals = []
        self.zero_outs = []
        for alloc in nc.m.functions[0].allocations:
            if not isinstance(alloc, mybir.MemoryLocationSet):
                continue
            name = alloc.memorylocations[0].name
            if alloc.kind == "ExternalInput":
                self.in_names.append(name)
            elif alloc.kind == "ExternalOutput":
                self.out_names.append(name)
                shape = tuple(alloc.tensor_shape)
                dt = mybir.dt.np(alloc.dtype)
                out_avals.append(jax.core.ShapedArray(shape, dt))
                self.zero_outs.append(np.zeros(shape, dt))
        all_in = tuple(self.in_names + self.out_names)
        n_params = len(self.in_names)
        out_names = tuple(self.out_names)
        out_avals_t = tuple(out_avals)

        def _body(*args):
            return tuple(
                _bass_exec_p.bind(
                    *args,
                    out_avals=out_avals_t,
                    in_names=all_in,
                    out_names=out_names,
                    lowering_input_output_aliases=(),
                    sim_require_finite=False,
                    sim_require_nnan=False,
                    nc=nc,
                )
            )

        self._jit = jax.jit(
            _body, donate_argnums=tuple(range(n_params, n_params + len(out_names)))
        )

    def start(self, in_map):
        import jax

        if self.pid_name is not None:
            in_map = {**in_map, self.pid_name: np.array([[self.core_id]], np.uint32)}
        with jax.default_device(self.device):
            args = [np.asarray(in_map[n]) for n in self.in_names] + [
                z.copy() for z in self.zero_outs
            ]
            return self._jit(*args)

    def finish(self, out_arrs):
        return {n: np.asarray(a) for n, a in zip(self.out_names, out_arrs)}


_RUNNERS = None
_RUNNER_META = None
_LOCK = threading.Lock()


def _get_runners(span_idxs):
    global _RUNNERS, _RUNNER_META
    key = span_idxs.tobytes()
    with _LOCK:
        if _RUNNERS is not None and _RUNNER_META[0] == key:
            return _RUNNERS, _RUNNER_META[1]
        import jax

        devs = jax.devices()[:B]
        plans = [_plan(*_spans(span_idxs[b, :, 0].astype(np.int64))) for b in range(B)]
        runners = []
        for b in range(B):
            nc = _build_program(plans[b])
            runners.append(CoreRunner(nc, devs[b], b))
        _RUNNERS = runners
        _RUNNER_META = (key, plans)
        return runners, plans


def kernel(input, lengths, span_idxs):
    input = np.asarray(input, dtype=np.float32)
    lengths = np.asarray(lengths, dtype=np.int32)
    span_idxs = np.asarray(span_idxs, dtype=np.int32)

    runners, plans = _get_runners(span_idxs)
    in_maps = _pack_inputs(input, plans)

    pending = [None] * B

    def launch(b):
        pending[b] = runners[b].start(in_maps[b])

    threads = [threading.Thread(target=launch, args=(b,)) for b in range(B)]
    for t in threads:
        t.start()
    for t in threads:
        t.join()

    out = np.zeros((B, S, 3 * D), np.float32)
    for b in range(B):
        out[b] = _unpack(runners[b].finish(pending[b]), plans[b])

    valid = ~((span_idxs[..., 0] == 0) & (span_idxs[..., 1] == 0)) & (
        np.arange(S)[None, :] < lengths[:, None]
    )
    out[~valid] = 0.0
    return out


# revision 51
# speedup vs baseline: 1.6259x; 1.0434x over previous
"""Segment-reduce (min/max/mean per contiguous span) on 8 Trainium2 cores.

Sharding: pure data parallel -- core b handles batch b. Programs are
specialized at build time on the span structure (span_idxs is host data).

Per-core algorithm (v2.1, fold-bucket design):

- min/max: host pads every span to lam = 2^ceil(lg L) rows (pad = repeat of
  the span's first element, neutral for min/max) and lays spans out in
  per-lam (sub-)buckets [lam, 4chunk, n_spans] in a feature-major layout
  (partition p = d % 128, chunk c = d // 128, bf16). Each sub-bucket is one
  DMA piece and one independent tensor_tensor fold-tree chain (bf16 2x DVE
  mode, 0.52 ns/elem), split between DVE and GPSIMD. Big chains stop at
  2-row remnants collected in a shared R2 array; one final TT per stat
  finishes all of them at once. Final values land contiguously in bucket
  order; the host un-permutes. No masks, no scans, no per-span extraction.
- sum/mean: TensorE matmul. lhsT = packed one-hot [128 tok, spans_in_tile]
  (fp8, ~10 cols per K-tile), rhs = x^T tile [128 tok, 512 d] (fp8),
  accumulating seg-sums in PSUM [s, d] (two banks for s 0-127 / 128-255,
  pre-zeroed by DVE). ACT scales by per-partition 1/L (activation Copy
  with scale vector) straight out of PSUM.
- spans with L <= 8 additionally get an exact bf16 fold-sum (fp8 error on
  tiny spans could breach tolerance): sum-fold over the padded rows, minus
  a host correction (lam-L)*x[start], times 1/L. Host takes mean for these
  spans from this path.

Outputs are bf16 (tolerance 2e-2); the host reassembles/permutes/casts.

Execution: each specialized program runs on its own NeuronCore via the
PJRT custom-call primitive (run_bass_via_pjrt's single-core path).
"""

import sys
import threading

sys.path.insert(0, "/opt/trn_rl_repo")

import numpy as np

B, T, D, S = 8, 4096, 512, 256
NK = T // 128  # matmul K-tiles
SUB_MAX = 4800  # max per-partition elems in one sub-bucket (DMA piece ~1.2MB)
GP_TARGET = 0  # fold elems (2 stats) assigned to GPSIMD (0 = GP disabled)
GP_EXTRA_LAMS = ()  # additional lam groups folded on GPSIMD
MEAN_ON_GP = False  # mean = psum * 1/L on GPSIMD instead of DVE


def _spans(span_starts):
    starts = span_starts.astype(np.int64)
    ends = np.empty_like(starts)
    ends[:-1] = starts[1:] - 1
    ends[-1] = T - 1
    return starts, ends


def _plan(starts, ends):
    """Bucket layout, sub-splitting, engine assignment, K-tile packing."""
    L = ends - starts + 1
    lam = np.maximum(2, 2 ** np.ceil(np.log2(np.maximum(L, 1))).astype(int))

    groups = {}
    for l in sorted(set(lam.tolist()), reverse=True):
        groups[l] = np.where(lam == l)[0]

    # GPSIMD takes the big lam-group whose 2-stat fold work (to 8-row
    # remnants) is closest to GP_TARGET. (GP custom tensor ops are not
    # supported by the axon lowering -- keep disabled until they are.)
    gp_lam = None
    best = None
    if GP_TARGET > 0:
        for l, spans in groups.items():
            if l < 16:
                continue
            work = 2 * 4 * len(spans) * (l - 8)
            score = abs(work - GP_TARGET)
            if best is None or score < best:
                best = score
                gp_lam = l

    # sub-bucket splitting; GP's first sub is kept small so GPSIMD can
    # start folding as soon as the first (small) DMA piece lands
    def make_subs(l, spans, gp):
        n = len(spans)
        if n == 0:
            return []
        subs = []
        i0 = 0
        if gp and n > 12:
            subs.append(spans[:8])
            i0 = 8
        max_n = max(2, SUB_MAX // (l * 4))
        rem = n - i0
        nsub = (rem + max_n - 1) // max_n
        per = (rem + nsub - 1) // nsub if nsub else rem
        for i in range(i0, n, per):
            subs.append(spans[i : i + per])
        # n >= 8 for big subs: keeps every fold width >= 64 elements
        # (narrower DVE tensor_tensor ops misbehave on this backend)
        return [
            dict(
                lam=l,
                spans=sp,
                nreal=len(sp),
                n=max(len(sp) + (len(sp) % 2), 8 if l >= 16 else 2),
                big=(l >= 16),
                gp=gp,
            )
            for sp in subs
        ]

    gp_lams = {gp_lam} | set(GP_EXTRA_LAMS) if gp_lam else set(GP_EXTRA_LAMS)
    gp_subs = []
    small_subs = []
    dve_big_subs = []
    for l, spans in groups.items():
        if l >= 16 and l in gp_lams:
            gp_subs.extend(make_subs(l, spans, True))
        elif l >= 16:
            dve_big_subs.extend(make_subs(l, spans, False))
        else:
            small_subs.extend(make_subs(l, spans, False))

    # APAD / DMA-piece order: GP data first, then smalls, then DVE bigs.
    order = gp_subs + small_subs + dve_big_subs
    off = 0
    for sb_ in order:
        sb_["off"] = off
        off += sb_["lam"] * 4 * sb_["n"]
    W = off

    # output columns: bigs (R2 order = their order in `order`), then smalls
    bigs = [s for s in order if s["big"]]
    smalls = [s for s in order if not s["big"]]
    NB = sum(s["n"] for s in bigs)
    NS = sum(s["n"] for s in smalls)
    SW = NB + NS
    col = 0
    for s in bigs:
        s["col"] = col  # also its R2 column offset
        col += s["n"]
    scol = 0
    for s in smalls:
        s["col"] = NB + scol
        s["s_off"] = scol
        scol += s["n"]
    perm = np.full(SW, -1, np.int64)
    for s in order:
        perm[s["col"] : s["col"] + s["nreal"]] = s["spans"]
    sperm = perm[NB:]

    # DMA pieces: one per big sub; all smalls together.
    # Transfer order (= SP issue order): interleave GP/DVE data so both
    # engines start early; AT (issued by ACT) lands mid-stream.
    pieces = []
    for s in gp_subs:
        pieces.append([s])
    if smalls:
        pieces.append(list(smalls))
    for s in dve_big_subs:
        pieces.append([s])
    for i, pc in enumerate(pieces):
        for s in pc:
            s["piece"] = i
    # issue order (sim-tuned): smallest DVE big group first (earliest DVE
    # start), then GP subs + smalls, then remaining groups by elems desc
    gsz = {}
    for s in dve_big_subs:
        gsz[s["lam"]] = gsz.get(s["lam"], 0) + s["lam"] * 4 * s["n"]
    issue = []
    if gsz:
        lmin = min(gsz, key=gsz.get)
        for s in dve_big_subs:
            if s["lam"] == lmin:
                issue.append(s["piece"])
    for s in gp_subs:
        issue.append(s["piece"])
    if smalls:
        issue.append(smalls[0]["piece"])
    for s in sorted(dve_big_subs, key=lambda s: -gsz[s["lam"]]):
        issue.append(s["piece"])
    seen = set()
    issue = [i for i in issue if not (i in seen or seen.add(i))]

    # token -> span id; K-tile one-hot packing (spans are the matmul free
    # dim, so no alignment constraints)
    seg = np.searchsorted(starts, np.arange(T), side="right") - 1
    ktiles = []
    oh_off = 0
    for q in range(NK):
        s_lo = int(seg[128 * q])
        s_hi = int(seg[128 * q + 127])
        m = s_hi - s_lo + 1
        ktiles.append(dict(s_lo=s_lo, m=m, off=oh_off))
        oh_off += m
    OHW = oh_off

    return dict(
        starts=starts,
        ends=ends,
        L=L,
        lam=lam,
        seg=seg,
        order=order,
        pieces=pieces,
        issue=issue,
        at_gate=issue[min(3, len(issue) - 1)],
        gp_subs=gp_subs,
        small_subs=smalls,
        dve_big_subs=dve_big_subs,
        W=W,
        NB=NB,
        NS=NS,
        SW=SW,
        perm=perm,
        sperm=sperm,
        ktiles=ktiles,
        OHW=OHW,
    )


def _build_program(plan):
    import concourse.bass as bass
    import concourse.mybir as mybir

    f32 = mybir.dt.float32
    bf16 = mybir.dt.bfloat16
    fp8 = mybir.dt.float8e4
    Alu = mybir.AluOpType
    Act = mybir.ActivationFunctionType
    nc = bass.Bass(target_bir_lowering=False)

    W, NB, NS, SW, OHW = plan["W"], plan["NB"], plan["NS"], plan["SW"], plan["OHW"]
    ktiles = plan["ktiles"]
    pieces = plan["pieces"]
    gp_subs = plan["gp_subs"]
    smalls = plan["small_subs"]
    dve_bigs = plan["dve_big_subs"]

    APAD = nc.dram_tensor("APAD", [128, W], bf16, kind="ExternalInput")
    AT = nc.dram_tensor("AT", [128, NK * D], fp8, kind="ExternalInput")
    OH = nc.dram_tensor("OH", [128, OHW], fp8, kind="ExternalInput")
    RC = nc.dram_tensor("RC", [128, S], bf16, kind="ExternalInput")
    CORRS = nc.dram_tensor("CORRS", [128, 8 * NS], bf16, kind="ExternalInput")
    # OUT planes: [min 4*SW | max 4*SW | smean 4*NS | mean 4*S], all d-major
    O_MIN, O_MAX = 0, 4 * SW
    O_SMEAN = 8 * SW
    O_ME = 8 * SW + 4 * NS
    OUTW = O_ME + 4 * S
    OUT = nc.dram_tensor("OUT", [128, OUTW], bf16, kind="ExternalOutput")

    from contextlib import ExitStack

    with ExitStack() as ctx:
        block = ctx.enter_context(nc.Block())
        sem = lambda n: ctx.enter_context(nc.semaphore(n))
        sb = lambda n, shape, dt: ctx.enter_context(nc.sbuf_tensor(n, shape, dt))

        psems = [sem(f"p{i}_sem") for i in range(len(pieces))]
        at_sems = [sem("at0_sem"), sem("at1_sem")]
        oh_sem = sem("oh_sem")
        rc_sem = sem("rc_sem")
        cs_sem = sem("cs_sem")
        psum_sem = sem("psum_sem")
        gp_rem = [sem("gp_rem0"), sem("gp_rem1")]
        min_done = sem("min_done")
        max_done = sem("max_done")
        mean_done = sem("mean_done")
        smean_done = sem("smean_done")
        o_sem = sem("o_sem")

        APAD_sb = sb("APAD_sb", [128, W], bf16)
        AT_sb = sb("AT_sb", [128, NK * D], fp8)
        OH_sb = sb("OH_sb", [128, OHW], fp8)
        RC_sb = sb("RC_sb", [128, S], bf16)
        CORRS_sb = sb("CORRS_sb", [128, 8 * NS], bf16)
        OUT_sb = sb("OUT_sb", [128, OUTW], bf16)
        R2 = [sb(f"R2_{s}", [128, 2 * 4 * NB], bf16) for s in range(2)]
        SS = sb("SS", [128, 4 * NS], bf16)
        SS2 = sb("SS2", [128, 4 * NS], bf16)
        P0 = ctx.enter_context(nc.psum_tensor("P0", [128, 512], f32))
        P1 = ctx.enter_context(nc.psum_tensor("P1", [128, 512], f32))

        # per-engine ping-pong fold scratch (reused chain to chain)
        def pool_sizes(subs, floor):
            a = b = 0
            for s_ in subs:
                szs = []
                rows = s_["lam"] // 2
                while rows >= floor:
                    szs.append(rows * 4 * s_["n"])
                    rows //= 2
                for i, sz in enumerate(szs):
                    if i % 2 == 0:
                        a = max(a, sz)
                    else:
                        b = max(b, sz)
            return a, b

        da, db = pool_sizes(dve_bigs + smalls, 1)
        da = max(da, 8 * 4 * max((s["n"] for s in gp_subs), default=0) // 2)
        ga, gb = pool_sizes(gp_subs, 8)
        DP = [sb("dpoolA", [128, max(da, 4)], bf16), sb("dpoolB", [128, max(db, 4)], bf16)]
        GPP = [sb("gpoolA", [128, max(ga, 4)], bf16), sb("gpoolB", [128, max(gb, 4)], bf16)]
        # persistent 8-row remnants for GP subs (read later by DVE)
        R8s = {}
        for sg in range(2):
            for i, s_ in enumerate(gp_subs):
                R8s[(sg, i)] = sb(f"r8_{sg}_{i}", [128, 8 * 4 * s_["n"]], bf16)

        def bview(s_):
            return (
                APAD_sb[:, s_["off"] : s_["off"] + s_["lam"] * 4 * s_["n"]]
                .rearrange("p (j c n) -> p j c n", j=s_["lam"], c=4)
            )

        def r2v(sg):
            return R2[sg][:].rearrange("p (j c n) -> p j c n", j=2, c=4)

        def out_cols(base, width, cols, n):
            return (
                OUT_sb[:, base : base + 4 * width]
                .rearrange("p (c w) -> p c w", c=4)[:, :, cols : cols + n]
            )

        OPS = {0: Alu.min, 1: Alu.max, 2: Alu.add}

        def fold_chain(eng, sg, s_, pool, cur=None, rows=None, stop_rows=None):
            """Fold [rows,4,n] by halving. Returns last instr.

            stop_rows=2 big chains write their last level into R2 columns;
            stop_rows=8 (GP) writes into the sub's persistent R8s buffer;
            stop_rows=1 (smalls / smallsum) writes OUT / SS.
            """
            op = OPS[sg]
            n = s_["n"]
            rw = 4 * n  # row width (elements) -- rows are contiguous
            if cur is None:
                cur = APAD_sb[:, s_["off"] : s_["off"] + s_["lam"] * rw]
                rows = s_["lam"]
            last = None
            pi = 0
            while rows > stop_rows:
                h = rows // 2
                in0 = cur[:, : h * rw]
                in1 = cur[:, h * rw : 2 * h * rw]
                if h == stop_rows and stop_rows == 2:
                    dst = r2v(sg)[:, :, :, s_["col"] : s_["col"] + n]
                    in0 = in0.rearrange("p (j c n) -> p j c n", j=2, c=4)
                    in1 = in1.rearrange("p (j c n) -> p j c n", j=2, c=4)
                elif h == stop_rows and stop_rows == 8:
                    dst = R8s[(sg, s_["gpi"])][:, : h * rw]
                elif h == 1:
                    if sg == 2:
                        dst = SS[:].rearrange("p (c n) -> p c n", c=4)[
                            :, :, s_["s_off"] : s_["s_off"] + n
                        ]
                    else:
                        dst = out_cols(O_MIN if sg == 0 else O_MAX, SW, s_["col"], n)
                    in0 = in0.rearrange("p (c n) -> p c n", c=4)
                    in1 = in1.rearrange("p (c n) -> p c n", c=4)
                else:
                    dst = pool[pi % 2][:, : h * rw]
                    pi += 1
                last = eng.tensor_tensor(dst, in0, in1, op)
                cur = dst
                rows = h
            return last

        for i, s_ in enumerate(gp_subs):
            s_["gpi"] = i

        @block.sync
        def _(sy):
            for i in plan["issue"]:
                pc = pieces[i]
                lo = pc[0]["off"]
                hi = pc[-1]["off"] + pc[-1]["lam"] * 4 * pc[-1]["n"]
                sy.dma_start(APAD_sb[:, lo:hi], APAD[:, lo:hi]).then_inc(psems[i], 16)

        @block.scalar
        def _(sc):
            sc.dma_start(OH_sb[:], OH[:]).then_inc(oh_sem, 16)
            sc.dma_start(RC_sb[:], RC[:]).then_inc(rc_sem, 16)
            sc.dma_start(CORRS_sb[:], CORRS[:]).then_inc(cs_sem, 16)
            # gate AT behind the early APAD pieces (PE only needs psum late)
            sc.wait_ge(psems[plan["at_gate"]], 16)
            sc.dma_start(AT_sb[:, : 16 * D], AT[:, : 16 * D]).then_inc(at_sems[0], 16)
            sc.dma_start(AT_sb[:, 16 * D :], AT[:, 16 * D :]).then_inc(at_sems[1], 16)
            sc.wait_ge(smean_done, 1)
            sc.dma_start(
                OUT[:, O_SMEAN : O_SMEAN + 4 * NS],
                OUT_sb[:, O_SMEAN : O_SMEAN + 4 * NS],
            ).then_inc(o_sem, 16)
            sc.wait_ge(min_done, 1)
            sc.dma_start(
                OUT[:, O_MIN : O_MIN + 4 * SW], OUT_sb[:, O_MIN : O_MIN + 4 * SW]
            ).then_inc(o_sem, 16)
            sc.wait_ge(mean_done, 1)
            sc.dma_start(
                OUT[:, O_ME : O_ME + 4 * S], OUT_sb[:, O_ME : O_ME + 4 * S]
            ).then_inc(o_sem, 16)
            sc.wait_ge(max_done, 1)
            sc.dma_start(
                OUT[:, O_MAX : O_MAX + 4 * SW], OUT_sb[:, O_MAX : O_MAX + 4 * SW]
            ).then_inc(o_sem, 16)
            sc.wait_ge(o_sem, 64)

        @block.tensor
        def _(pe):
            pe.wait_ge(oh_sem, 16)
            for half in range(2):
                pe.wait_ge(at_sems[half], 16)
                for q in range(16 * half, 16 * half + 16):
                    kt = ktiles[q]
                    for c in range(4):
                        P = P0 if c < 2 else P1
                        coloff = 256 * (c % 2)
                        is_last = q == NK - 1 and c % 2 == 1
                        mm = nc.tensor.matmul(
                            P[:, coloff + kt["s_lo"] : coloff + kt["s_lo"] + kt["m"]],
                            AT_sb[:, D * q + 128 * c : D * q + 128 * (c + 1)],
                            OH_sb[:, kt["off"] : kt["off"] + kt["m"]],
                            start=(q == 0 and c % 2 == 0),
                            stop=is_last,
                            skip_group_check=True,
                        )
                        if is_last:
                            mm.then_inc(psum_sem, 1)

        def emit_mean(eng):
            eng.wait_ge(psum_sem, 2)
            eng.wait_ge(rc_sem, 16)
            for c in range(4):
                P = P0 if c < 2 else P1
                coloff = 256 * (c % 2)
                last = eng.tensor_tensor(
                    out_cols(O_ME, S, 0, S)[:, c],
                    P[:, coloff : coloff + S],
                    RC_sb[:],
                    Alu.mult,
                )
            return last

        @block.gpsimd
        def _(g):
            for sg in range(2):
                last = None
                for s_ in gp_subs:
                    g.wait_ge(psems[s_["piece"]], 16)
                    last = fold_chain(g, sg, s_, GPP, stop_rows=8)
                if last is not None:
                    last.then_inc(gp_rem[sg], 1)
                else:
                    g.sem_inc(gp_rem[sg], 1)
            if MEAN_ON_GP:
                emit_mean(g).then_inc(mean_done, 1)

        @block.vector
        def _(v):
            # process pieces in DMA issue order; min+max per piece so late
            # pieces don't block early work
            for i in plan["issue"]:
                pc = pieces[i]
                if not pc[0]["big"]:
                    v.wait_ge(psems[i], 16)
                    for sg in (0, 1, 2):
                        for s_ in pc:
                            fold_chain(v, sg, s_, DP, stop_rows=1)
                    v.wait_ge(cs_sem, 16)
                    v.tensor_tensor(
                        SS2[:], SS[:], CORRS_sb[:, : 4 * NS], Alu.subtract
                    )
                    v.tensor_tensor(
                        OUT_sb[:, O_SMEAN : O_SMEAN + 4 * NS],
                        SS2[:],
                        CORRS_sb[:, 4 * NS :],
                        Alu.mult,
                    ).then_inc(smean_done, 1)
                elif not pc[0]["gp"]:
                    v.wait_ge(psems[i], 16)
                    for s_ in pc:
                        fold_chain(v, 0, s_, DP, stop_rows=2)
                        fold_chain(v, 1, s_, DP, stop_rows=2)
            # finish GP remnants 8 -> 2 into R2, then one final TT per stat
            for sg in (0, 1):
                if gp_subs:
                    v.wait_ge(gp_rem[sg], 1)
                    for s_ in gp_subs:
                        fold_chain(
                            v, sg, s_, DP,
                            cur=R8s[(sg, s_["gpi"])][:],
                            rows=8,
                            stop_rows=2,
                        )
                if NB:
                    base = O_MIN if sg == 0 else O_MAX
                    v.tensor_tensor(
                        out_cols(base, SW, 0, NB), r2v(sg)[:, 0], r2v(sg)[:, 1], OPS[sg]
                    )
                v.drain()
                v.sem_inc(min_done if sg == 0 else max_done, 1)

            # mean = psum * (1/L): [128 d, S] per chunk
            if not MEAN_ON_GP:
                emit_mean(v)
                v.drain()
                v.sem_inc(mean_done, 1)

    return nc


def _pack_inputs(input, plans):
    import ml_dtypes

    bf16 = ml_dtypes.bfloat16
    try:
        fp8 = ml_dtypes.float8_e4m3
    except AttributeError:
        fp8 = ml_dtypes.float8_e4m3fn

    in_maps = []
    for b in range(B):
        x = input[b]  # [T, D] f32
        plan = plans[b]
        W, NS, OHW = plan["W"], plan["NS"], plan["OHW"]
        starts, L = plan["starts"], plan["L"]

        APAD = np.zeros((128, W), np.float32)
        for bk in plan["order"]:
            lamk, n, nreal = bk["lam"], bk["n"], bk["nreal"]
            spans = bk["spans"]
            j = np.arange(lamk)
            tok = np.where(
                j[None, :] < L[spans][:, None],
                starts[spans][:, None] + j[None, :],
                starts[spans][:, None],
            )
            arr = x[tok]  # [nreal, lam, D]
            arr = arr.reshape(nreal, lamk, 4, 128).transpose(3, 1, 2, 0)
            dst = APAD[:, bk["off"] : bk["off"] + lamk * 4 * n].reshape(
                128, lamk, 4, n
            )
            dst[:, :, :, :nreal] = arr
        APAD = APAD.astype(bf16)

        AT = np.ascontiguousarray(
            x.reshape(NK, 128, D).transpose(1, 0, 2).reshape(128, NK * D)
        ).astype(fp8)

        OHm = np.zeros((128, OHW), np.float32)
        seg = plan["seg"]
        t = np.arange(128)
        for q, kt in enumerate(plan["ktiles"]):
            OHm[t, kt["off"] + seg[128 * q + t] - kt["s_lo"]] = 1.0
        OHm = OHm.astype(fp8)

        RC = np.ascontiguousarray(
            np.broadcast_to((1.0 / L.astype(np.float32))[None, :], (128, S))
        ).astype(bf16)

        CORRS = np.zeros((128, 8 * NS), np.float32)
        corr = CORRS[:, : 4 * NS].reshape(128, 4, NS)
        rcs = CORRS[:, 4 * NS :].reshape(128, 4, NS)
        for bk in plan["small_subs"]:
            spans = bk["spans"]
            pad = (bk["lam"] - L[spans]).astype(np.float32)
            x0 = x[starts[spans]]  # [nreal, D]
            cc = (pad[:, None] * x0).reshape(-1, 4, 128).transpose(2, 1, 0)
            sl = slice(bk["s_off"], bk["s_off"] + bk["nreal"])
            corr[:, :, sl] = cc
            rcs[:, :, sl] = (1.0 / L[spans].astype(np.float32))[None, None, :]
        CORRS = CORRS.astype(bf16)

        in_maps.append({"APAD": APAD, "AT": AT, "OH": OHm, "RC": RC, "CORRS": CORRS})
    return in_maps


def _unpack(res_b, plan):
    NB, NS, SW = plan["NB"], plan["NS"], plan["SW"]
    O = res_b["OUT"].astype(np.float32)
    O_MIN, O_MAX = 0, 4 * SW
    O_SMEAN = 8 * SW
    O_ME = 8 * SW + 4 * NS

    def plane(base, width):
        return (
            O[:, base : base + 4 * width]
            .reshape(128, 4, width)
            .transpose(2, 1, 0)
            .reshape(width, D)
        )

    out = np.zeros((S, 3 * D), np.float32)
    perm = plan["perm"]
    valid = perm >= 0
    out[perm[valid], 0:D] = plane(O_MIN, SW)[valid]
    out[perm[valid], D : 2 * D] = plane(O_MAX, SW)[valid]
    out[:, 2 * D :] = plane(O_ME, S)
    if NS:
        sperm = plan["sperm"]
        sv = sperm >= 0
        out[sperm[sv], 2 * D :] = plane(O_SMEAN, NS)[sv]
    return out


class CoreRunner:
    """jit-once runner for one specialized program on one NeuronCore."""

    def __init__(self, nc, device, core_id):
        import jax
        import concourse.mybir as mybir
        from concourse.bass2jax import install_neuronx_cc_hook, _bass_exec_p

        install_neuronx_cc_hook()
        self.device = device
        self.core_id = core_id
        self.pid_name = (
            nc.partition_id_tensor.name if nc.partition_id_tensor is not None else None
        )
        self.in_names = []
        self.out_names = []
        out_avals = []
        self.zero_outs = []
        for alloc in nc.m.functions[0].allocations:
            if not isinstance(alloc, mybir.MemoryLocationSet):
                continue
            name = alloc.memorylocations[0].name
            if alloc.kind == "ExternalInput":
                self.in_names.append(name)
            elif alloc.kind == "ExternalOutput":
                self.out_names.append(name)
                shape = tuple(alloc.tensor_shape)
                dt = mybir.dt.np(alloc.dtype)
                out_avals.append(jax.core.ShapedArray(shape, dt))
                self.zero_outs.append(np.zeros(shape, dt))
        all_in = tuple(self.in_names + self.out_names)
        n_params = len(self.in_names)
        out_names = tuple(self.out_names)
        out_avals_t = tuple(out_avals)

        def _body(*args):
            return tuple(
                _bass_exec_p.bind(
                    *args,
                    out_avals=out_avals_t,
                    in_names=all_in,
                    out_names=out_names,
                    lowering_input_output_aliases=(),
                    sim_require_finite=False,
                    sim_require_nnan=False,
                    nc=nc,
                )
            )

        self._jit = jax.jit(
            _body, donate_argnums=tuple(range(n_params, n_params + len(out_names)))
        )

    def start(self, in_map):
        import jax

        if self.pid_name is not None:
            in_map = {**in_map, self.pid_name: np.array([[self.core_id]], np.uint32)}
        with jax.default_device(self.device):
            args = [np.asarray(in_map[n]) for n in self.in_names] + [
                z.copy() for z in self.zero_outs
            ]
            return self._jit(*args)

    def finish(self, out_arrs):
        return {n: np.asarray(a) for n, a in zip(self.out_names, out_arrs)}


_RUNNERS = None
_RUNNER_META = None
_LOCK = threading.Lock()


def _get_runners(span_idxs):
    global _RUNNERS, _RUNNER_META
    key = span_idxs.tobytes()
    with _LOCK:
        if _RUNNERS is not None and _RUNNER_META[0] == key:
            return _RUNNERS, _RUNNER_META[1]
        import jax

        devs = jax.devices()[:B]
        plans = [_plan(*_spans(span_idxs[b, :, 0].astype(np.int64))) for b in range(B)]
        runners = []
        for b in range(B):
            nc = _build_program(plans[b])
            runners.append(CoreRunner(nc, devs[b], b))
        _RUNNERS = runners
        _RUNNER_META = (key, plans)
        return runners, plans


def kernel(input, lengths, span_idxs):
    input = np.asarray(input, dtype=np.float32)
    lengths = np.asarray(lengths, dtype=np.int32)
    span_idxs = np.asarray(span_idxs, dtype=np.int32)

    runners, plans = _get_runners(span_idxs)
    in_maps = _pack_inputs(input, plans)

    pending = [None] * B

    def launch(b):
        pending[b] = runners[b].start(in_maps[b])

    threads = [threading.Thread(target=launch, args=(b,)) for b in range(B)]
    for t in threads:
        t.start()
    for t in threads:
        t.join()

    out = np.zeros((B, S, 3 * D), np.float32)
    for b in range(B):
        out[b] = _unpack(runners[b].finish(pending[b]), plans[b])

    valid = ~((span_idxs[..., 0] == 0) & (span_idxs[..., 1] == 0)) & (
        np.arange(S)[None, :] < lengths[:, None]
    )
    out[~valid] = 0.0
    return out
